# revision 11
# baseline (speedup 1.0000x reference)
"""Fused multi-head attention block (QKV proj -> softmax attention -> out proj
-> residual LayerNorm, plus head-averaged attention map) on 8 Trainium2
NeuronCores.

Sharding: queries. Core c handles batch c//2, query rows (c%2)*1024..+1024,
all 16 heads locally, so avg_attention needs no collectives. Each core
recomputes K/V for its whole batch (2048 tokens). Host rolls each batch so
the core's queries are tokens 0..1023; avg columns are un-rolled after.

Per-core dataflow (T=2048 keys, TQ=1024 queries, 16 heads of 64):
  phase 1: KT=[D,T], QT=[D,TQ], V=[T,D] projections (f32r matmuls, PSUM
           accumulated over din), spilled to DRAM scratch.
  phase 2: two q-half passes; per head-pair p (row-tiled K=64 pairs on PE):
    s[q,k] scores -> ACT exp(scale) with accum_out -> Z col -> fused
        scalar_tensor_tensor: avg[qt] += exp * (1/Z)      (DVE, one pass)
    sT[k,q] scores -> ACT exp -> ctx^T = [1|V_h]^T @ expT (PSUM k-accum);
        ctx row 0 = Z row -> reciprocal/broadcast -> normalize ctx.
  phase 3: out rows = LayerNorm(ctx^T.T @ Wo + bo + xq) with Newton-refined
           rsqrt; per-partition mean/var via reduce_sum + ACT Square accum.
"""

import numpy as np
from contextlib import ExitStack

import concourse.bacc as bacc
import concourse.tile as tile
from concourse import mybir
from concourse.bass_utils import run_bass_kernel_spmd

F32 = mybir.dt.float32
MMT = mybir.dt.float32r  # PE full-rate fp32 (reduced-precision multiply)
AX = mybir.AxisListType.X
OP = mybir.AluOpType
ACTF = mybir.ActivationFunctionType

B, S, D, H = 4, 2048, 1024, 16
HD = D // H           # 64
T = S                 # keys per batch
TQ = S // 2           # queries per core
P = 128
NP = 8                # head pairs
QH = 512              # q-half width
SCALE = 1.0 / (np.sqrt(HD) * 0.5)   # = 0.25
EPS = 1e-5


def build_program():
    nc = bacc.Bacc("TRN2", target_bir_lowering=False, debug=False)

    xT = nc.dram_tensor("xT", [D, T], MMT, kind="ExternalInput").ap()
    xq = nc.dram_tensor("xq", [TQ, D], F32, kind="ExternalInput").ap()
    Wq = nc.dram_tensor("Wq", [D, D], MMT, kind="ExternalInput").ap()
    Wk = nc.dram_tensor("Wk", [D, D], MMT, kind="ExternalInput").ap()
    Wv = nc.dram_tensor("Wv", [D, D], MMT, kind="ExternalInput").ap()
    Wo = nc.dram_tensor("Wo", [D, D], MMT, kind="ExternalInput").ap()
    bqc = nc.dram_tensor("bqc", [P, 8], F32, kind="ExternalInput").ap()
    bkc = nc.dram_tensor("bkc", [P, 8], F32, kind="ExternalInput").ap()
    bv = nc.dram_tensor("bv", [1, D], MMT, kind="ExternalInput").ap()
    bo = nc.dram_tensor("bo", [1, D], MMT, kind="ExternalInput").ap()
    lng = nc.dram_tensor("lng", [P, D], F32, kind="ExternalInput").ap()
    lnb = nc.dram_tensor("lnb", [P, D], F32, kind="ExternalInput").ap()
    out_r = nc.dram_tensor("out_r", [TQ, D], F32, kind="ExternalOutput").ap()
    avg_r = nc.dram_tensor("avg_r", [TQ, T], F32, kind="ExternalOutput").ap()

    with tile.TileContext(nc) as tc, ExitStack() as ctx:
        ep = ctx.enter_context

        dram = ep(tc.tile_pool(name="dram", bufs=1, space="DRAM"))
        KT_d = dram.tile([D, T], MMT, tag="KT_d")
        QT_d = dram.tile([D, TQ], MMT, tag="QT_d")
        V_d = dram.tile([T, H * 65], MMT, tag="V_d")
        ctxT_d = dram.tile([D, TQ], MMT, tag="ctxT_d")

        consts = ep(tc.tile_pool(name="consts", bufs=1))
        zrow = consts.tile([1, 512], F32, tag="zrow")
        nc.vector.memset(zrow[:], 0.0)
        ones_row = consts.tile([1, 512], MMT, tag="ones_row")
        nc.vector.tensor_scalar(ones_row[:], zrow[:], 1.0, None, op0=OP.add)
        z16 = consts.tile([P, 16], F32, tag="z16")
        nc.vector.memset(z16[:], 0.0)
        ones16 = consts.tile([P, 16], MMT, tag="ones16")
        nc.vector.tensor_scalar(ones16[:], z16[:], 1.0, None, op0=OP.add)
        bkc_sb = consts.tile([P, 8], F32, tag="bkc_sb")
        nc.sync.dma_start(bkc_sb[:], bkc)
        bqc_sb = consts.tile([P, 8], F32, tag="bqc_sb")
        nc.sync.dma_start(bqc_sb[:], bqc)
        bv_sb = consts.tile([1, D], MMT, tag="bv_sb")
        nc.sync.dma_start(bv_sb[:], bv)
        bo_sb = consts.tile([1, D], MMT, tag="bo_sb")
        nc.sync.dma_start(bo_sb[:], bo)
        lng_b = consts.tile([P, D], F32, tag="lng_b")
        nc.sync.dma_start(lng_b[:], lng)
        lnb_b = consts.tile([P, D], F32, tag="lnb_b")
        nc.sync.dma_start(lnb_b[:], lnb)

        ps512 = ep(tc.tile_pool(name="ps512", bufs=6, space="PSUM"))
        ps_ctx = ep(tc.tile_pool(name="ps_ctx", bufs=2, space="PSUM"))
        wres = ep(tc.tile_pool(name="wres", bufs=1))
        xs = ep(tc.tile_pool(name="xs", bufs=9))
        stg = ep(tc.tile_pool(name="stg", bufs=3))
        stgv = ep(tc.tile_pool(name="stgv", bufs=2))
        kqs = ep(tc.tile_pool(name="kqs", bufs=2))
        qqs = ep(tc.tile_pool(name="qqs", bufs=2))
        vvs = ep(tc.tile_pool(name="vvs", bufs=18))
        expu = ep(tc.tile_pool(name="expu", bufs=2))
        expt = ep(tc.tile_pool(name="expt", bufs=3))
        tiny = ep(tc.tile_pool(name="tiny", bufs=12))
        rowp = ep(tc.tile_pool(name="rowp", bufs=2))
        ctxs = ep(tc.tile_pool(name="ctxs", bufs=2))
        avgp = ep(tc.tile_pool(name="avgp", bufs=1))
        cxt = ep(tc.tile_pool(name="cxt", bufs=8))
        outsp = ep(tc.tile_pool(name="outsp", bufs=1))

        # ============ phase 1: projections ============
        def proj_T(W_ap, bcol, out_d, ntok):
            """out_d[D, ntok] = W^T @ xT[:, :ntok] + bias(col form)."""
            wt = [wres.tile([P, D], MMT, name=f"w{i}", tag=f"w{i}")
                  for i in range(8)]
            for i in range(8):
                nc.sync.dma_start(wt[i][:], W_ap[i * P:(i + 1) * P, :])
            for q4 in range(ntok // 512):
                xtq = []
                for din in range(8):
                    t_ = xs.tile([P, 512], MMT, tag="xs")
                    nc.sync.dma_start(
                        t_[:], xT[din * P:(din + 1) * P,
                                  q4 * 512:(q4 + 1) * 512])
                    xtq.append(t_)
                for dp in range(8):
                    ps = ps512.tile([P, 512], F32, tag="ps512")
                    for din in range(8):
                        nc.tensor.matmul(
                            ps[:], (wt[din][:, dp * P:(dp + 1) * P]),
                            (xtq[din][:]), start=(din == 0),
                            stop=(din == 7))
                    sb = stg.tile([P, 512], MMT, tag="stg")
                    nc.vector.tensor_scalar(
                        sb[:], ps[:], bcol[:, dp:dp + 1], None, op0=OP.add)
                    nc.sync.dma_start(
                        out_d[dp * P:(dp + 1) * P,
                              q4 * 512:(q4 + 1) * 512], sb[:])

        def proj_V():
            wt = [wres.tile([P, D], MMT, name=f"w{i}", tag=f"w{i}")
                  for i in range(8)]
            for i in range(8):
                nc.sync.dma_start(wt[i][:], Wv[i * P:(i + 1) * P, :])
            for q4 in range(4):
                xtq = []
                for din in range(8):
                    t_ = xs.tile([P, 512], MMT, tag="xs")
                    nc.sync.dma_start(
                        t_[:], xT[din * P:(din + 1) * P,
                                  q4 * 512:(q4 + 1) * 512])
                    xtq.append(t_)
                for tt in range(4):
                    sb2 = stgv.tile([P, H * 65], MMT, tag="stgv")
                    sbv = sb2[:].rearrange("p (g c) -> p g c", g=H, c=65)
                    for dc in range(2):
                        ps = ps512.tile([P, 512], F32, tag="ps512")
                        for din in range(8):
                            nc.tensor.matmul(
                                ps[:], (xtq[din][:, tt * P:(tt + 1) * P]),
                                (wt[din][:, dc * 512:(dc + 1) * 512]),
                                start=(din == 0), stop=False)
                        nc.tensor.matmul(
                            ps[:], (ones_row[:, 0:P]),
                            (bv_sb[:, dc * 512:(dc + 1) * 512]),
                            start=False, stop=True)
                        nc.vector.tensor_copy(
                            sbv[:, dc * 8:(dc + 1) * 8, 0:64],
                            ps[:].rearrange("p (g c) -> p g c", g=8, c=64))
                    nc.vector.tensor_copy(sbv[:, :, 64], ones16[:])
                    tok0 = q4 * 512 + tt * P
                    nc.sync.dma_start(V_d[tok0:tok0 + P, :], sb2[:])

        proj_T(Wk, bkc_sb, KT_d, T)
        proj_T(Wq, bqc_sb, QT_d, TQ)
        proj_V()

        # ============ phase 2: attention, two q-half passes ============
        for qhf in range(2):
            q0 = qhf * QH
            avg = [avgp.tile([P, T], F32, name=f"avg{i}", tag=f"avg{i}")
                   for i in range(4)]
            for a in avg:
                nc.vector.memset(a[:], 0.0)
            for p in range(NP):
                ktp = kqs.tile([P, T], MMT, tag="kqs")
                nc.sync.dma_start(ktp[:], KT_d[p * P:(p + 1) * P, :])
                qtp = qqs.tile([P, QH], MMT, tag="qqs")
                nc.sync.dma_start(qtp[:], QT_d[p * P:(p + 1) * P,
                                               q0:q0 + QH])
                vts = []
                for kt in range(16):
                    vt = vvs.tile([P, 130], MMT, tag="vvs")
                    nc.sync.dma_start(
                        vt[:], V_d[kt * P:(kt + 1) * P,
                                   p * 130:(p + 1) * 130])
                    vts.append(vt)

                for h in range(2):
                    hp = h * HD
                    # ---- s path: per local q tile ----
                    for q4 in range(4):
                        qs = q4 * P  # offset within the half
                        eu = expu.tile([P, T], F32, tag="expu")
                        zp = tiny.tile([P, 4], F32, tag="zp")
                        for kc in range(4):
                            ps = ps512.tile([P, 512], F32, tag="ps512")
                            nc.tensor.matmul(
                                ps[:], (qtp[hp:hp + HD, qs:qs + P]),
                                (ktp[hp:hp + HD,
                                        kc * 512:(kc + 1) * 512]),
                                start=True, stop=True,
                                tile_position=(hp, 0))
                            nc.scalar.activation(
                                eu[:, kc * 512:(kc + 1) * 512], ps[:],
                                ACTF.Exp, scale=SCALE,
                                accum_out=zp[:, kc:kc + 1])
                        z = tiny.tile([P, 1], F32, tag="z")
                        nc.vector.reduce_sum(z[:], zp[:], axis=AX)
                        r = tiny.tile([P, 1], F32, tag="r")
                        nc.vector.reciprocal(r[:], z[:])
                        nc.vector.tensor_scalar_mul(r[:], r[:], 1.0 / H)
                        nc.vector.scalar_tensor_tensor(
                            avg[q4][:], eu[:], r[:], avg[q4][:],
                            op0=OP.mult, op1=OP.add)

                    # ---- sT path + ctx ----
                    pc = ps_ctx.tile([65, QH], F32, tag="ps_ctx")
                    for kt in range(16):
                        et = expt.tile([P, QH], MMT, tag="expt")
                        ps = ps512.tile([P, 512], F32, tag="ps512")
                        nc.tensor.matmul(
                            ps[:], (ktp[hp:hp + HD, kt * P:(kt + 1) * P]),
                            (qtp[hp:hp + HD, :]),
                            start=True, stop=True, tile_position=(hp, 0))
                        nc.scalar.activation(et[:], ps[:], ACTF.Exp,
                                             scale=SCALE)
                        nc.tensor.matmul(
                            pc[:], (vts[kt][:, h * 65:(h + 1) * 65]),
                            (et[:]), start=(kt == 0), stop=(kt == 15))
                    rr = rowp.tile([1, QH], F32, tag="rr")
                    nc.vector.reciprocal(rr[:], pc[64:65, :])
                    rb = rowp.tile([64, QH], F32, tag="rb")
                    nc.gpsimd.partition_broadcast(rb[:], rr[:])
                    cs = ctxs.tile([64, QH], MMT, tag="ctxs")
                    nc.vector.tensor_mul(cs[:], pc[0:64, :], rb[:])
                    d0 = (2 * p + h) * HD
                    nc.sync.dma_start(ctxT_d[d0:d0 + HD, q0:q0 + QH],
                                      cs[:])

            for q4 in range(4):
                nc.sync.dma_start(
                    avg_r[q0 + q4 * P:q0 + (q4 + 1) * P, :], avg[q4][:])

        # ============ phase 3: out proj + residual + LayerNorm ============
        wt = [wres.tile([P, D], MMT, name=f"w{i}", tag=f"w{i}")
                  for i in range(8)]
        for i in range(8):
            nc.sync.dma_start(wt[i][:], Wo[i * P:(i + 1) * P, :])
        for qg in range(4):
            cxts = []
            for dt in range(8):
                t_ = cxt.tile([P, 256], MMT, tag="cxt")
                nc.sync.dma_start(
                    t_[:], ctxT_d[dt * P:(dt + 1) * P,
                                  qg * 256:(qg + 1) * 256])
                cxts.append(t_)
            for qh2 in range(2):
                qt = qg * 2 + qh2
                xqt = outsp.tile([P, D], F32, tag="xqt")
                nc.sync.dma_start(xqt[:], xq[qt * P:(qt + 1) * P, :])
                h_sb = outsp.tile([P, D], F32, tag="h_sb")
                for dc in range(2):
                    ps = ps512.tile([P, 512], F32, tag="ps512")
                    for dt in range(8):
                        nc.tensor.matmul(
                            ps[:], (cxts[dt][:, qh2 * P:(qh2 + 1) * P]),
                            (wt[dt][:, dc * 512:(dc + 1) * 512]),
                            start=(dt == 0), stop=False)
                    nc.tensor.matmul(
                        ps[:], (ones_row[:, 0:P]),
                        (bo_sb[:, dc * 512:(dc + 1) * 512]),
                        start=False, stop=True)
                    nc.vector.tensor_add(
                        h_sb[:, dc * 512:(dc + 1) * 512], ps[:],
                        xqt[:, dc * 512:(dc + 1) * 512])
                mu = tiny.tile([P, 1], F32, tag="mu")
                nc.vector.reduce_sum(mu[:], h_sb[:], axis=AX)
                nc.vector.tensor_scalar_mul(mu[:], mu[:], 1.0 / D)
                o1 = outsp.tile([P, D], F32, tag="o1")
                ss = tiny.tile([P, 1], F32, tag="ss")
                nc.scalar.activation(o1[:], h_sb[:], ACTF.Square,
                                     accum_out=ss[:])
                var = tiny.tile([P, 1], F32, tag="var")
                nc.vector.tensor_scalar_mul(var[:], ss[:], 1.0 / D)
                mu2 = tiny.tile([P, 1], F32, tag="mu2")
                nc.vector.tensor_mul(mu2[:], mu[:], mu[:])
                nc.vector.tensor_sub(var[:], var[:], mu2[:])
                nc.vector.tensor_scalar_add(var[:], var[:], EPS)
                sd = tiny.tile([P, 1], F32, tag="sd")
                nc.scalar.sqrt(sd[:], var[:])
                rs = tiny.tile([P, 1], F32, tag="rs")
                nc.vector.reciprocal(rs[:], sd[:])
                t1 = tiny.tile([P, 1], F32, tag="t1")
                nc.vector.tensor_mul(t1[:], rs[:], rs[:])
                nc.vector.tensor_mul(t1[:], t1[:], var[:])
                nc.vector.tensor_scalar(t1[:], t1[:], -0.5, 1.5,
                                        op0=OP.mult, op1=OP.add)
                nc.vector.tensor_mul(rs[:], rs[:], t1[:])
                nc.vector.tensor_scalar(o1[:], h_sb[:], mu[:], rs[:],
                                        op0=OP.subtract, op1=OP.mult)
                nc.vector.tensor_mul(o1[:], o1[:], lng_b[:])
                nc.vector.tensor_add(o1[:], o1[:], lnb_b[:])
                nc.sync.dma_start(out_r[qt * P:(qt + 1) * P, :], o1[:])

    nc.compile()
    return nc


_NC_CACHE = None


def _get_nc():
    global _NC_CACHE
    if _NC_CACHE is None:
        _NC_CACHE = build_program()
    return _NC_CACHE


def make_in_maps(x, Wq, bq, Wk, bk, Wv, bv, Wo, bo, ln_g, ln_b):
    x = np.asarray(x, np.float32)
    shared = {
        "Wq": np.ascontiguousarray(Wq, np.float32),
        "Wk": np.ascontiguousarray(Wk, np.float32),
        "Wv": np.ascontiguousarray(Wv, np.float32),
        "Wo": np.ascontiguousarray(Wo, np.float32),
        "bqc": np.ascontiguousarray(
            np.asarray(bq, np.float32).reshape(8, P).T),
        "bkc": np.ascontiguousarray(
            np.asarray(bk, np.float32).reshape(8, P).T),
        "bv": np.asarray(bv, np.float32).reshape(1, D),
        "bo": np.asarray(bo, np.float32).reshape(1, D),
        "lng": np.ascontiguousarray(
            np.broadcast_to(np.asarray(ln_g, np.float32), (P, D))),
        "lnb": np.ascontiguousarray(
            np.broadcast_to(np.asarray(ln_b, np.float32), (P, D))),
    }
    in_maps = []
    for c in range(8):
        b = c // 2
        qoff = (c % 2) * TQ
        xb = x[b]
        xr = np.concatenate([xb[qoff:], xb[:qoff]], axis=0)
        in_maps.append({
            "xT": np.ascontiguousarray(xr.T),
            "xq": np.ascontiguousarray(xr[:TQ]),
            **shared,
        })
    return in_maps


def gather_results(results):
    out = np.empty((B, S, D), np.float32)
    avg = np.empty((B, S, S), np.float32)
    for c in range(8):
        b = c // 2
        qoff = (c % 2) * TQ
        out[b, qoff:qoff + TQ] = results[c]["out_r"]
        avg[b, qoff:qoff + TQ] = np.roll(results[c]["avg_r"], qoff, axis=1)
    return out, avg


def kernel(x, Wq, bq, Wk, bk, Wv, bv, Wo, bo, ln_g, ln_b):
    in_maps = make_in_maps(x, Wq, bq, Wk, bk, Wv, bv, Wo, bo, ln_g, ln_b)
    nc = _get_nc()
    res = run_bass_kernel_spmd(nc, in_maps, list(range(8))).results
    return gather_results(res)
